# revision 30
# baseline (speedup 1.0000x reference)
"""DiT block kernel for 8 Trainium2 NeuronCores.

Sharding: data-parallel over batch (B=8 -> one batch element per core).
Each core computes the full DiT block for its batch element.

Layout per core (L=1024 tokens, D=1024, H=16 heads, HD=64, FF=4096):
  - x resident token-major [128, 8tile, 1024] fp32 (residual stream, in-place)
  - LN via bn_stats/bn_aggr (tokens on partitions); modulation vectors
    broadcast once across partitions via gpsimd partition_broadcast
  - h1/h2 transposed feature-major via PE transposes (bf16, 1 cyc/row)
  - Q^T/K^T feature-major; V token-major; softmax denominator via a
    col-packed ones matmul running concurrently with attn@V (flash-style)
  - scores computed transposed (keys on partitions, 2 heads row-packed)
    so no 16M-element attention transpose is needed
  - matmul operands bf16 (fp32 PSUM accumulation), residual stream fp32
  - MLP work of chunk qc is software-pipelined into the attention of chunk
    qc+1 so the TensorEngine stays dense (HAM stays warm) while the
    Activation engine drains the exp() stream
"""

import sys

sys.path.insert(0, "/opt/trn_rl_repo")

from collections import deque

import numpy as np
import ml_dtypes

import concourse.bacc as bacc
import concourse.tile as tile
from concourse import mybir
from concourse import bass_utils
from concourse.masks import make_identity

F32 = mybir.dt.float32
BF16 = mybir.dt.bfloat16
BF = ml_dtypes.bfloat16
OP = mybir.AluOpType
AF = mybir.ActivationFunctionType

B = 8
L = 1024
D = 1024
H = 16
HD = 64
FF = 4096
LN_EPS = 1e-5
P = 128
TT = L // P          # 8 token tiles
KT = D // P          # 8 feature tiles
FT = FF // P         # 32 ff tiles
QC = 256             # query-chunk size
NQC = L // QC        # 4 query chunks
QSUB = QC // P       # 2 token tiles per query chunk

_CACHE = {}


def _build():
    nc = bacc.Bacc(None, target_bir_lowering=False)
    names = {}
    with tile.TileContext(nc) as tc, \
            tc.tile_pool(name="dram", bufs=1, space="DRAM") as dram, \
            tc.tile_pool(name="per", bufs=1) as per, \
            tc.tile_pool(name="ps_t", bufs=2, space="PSUM") as ps_t:

        # all inputs pre-rearranged host-side to partition-major layouts so
        # every DMA is contiguous per partition
        x_d = dram.tile([P, TT, D], F32, kind="ExternalInput", name="x")
        c_d = dram.tile([P, KT], F32, kind="ExternalInput", name="c")
        ln_d = dram.tile([4, D], BF16, kind="ExternalInput", name="ln")
        adaw_d = dram.tile([P, KT, 6 * D], BF16, kind="ExternalInput", name="ada_w")
        adab_d = dram.tile([1, 6 * D], F32, kind="ExternalInput", name="ada_b")
        qkvw_d = dram.tile([P, KT, 3 * D], BF16, kind="ExternalInput", name="qkv_w")
        qkb_d = dram.tile([P, 2 * KT], F32, kind="ExternalInput", name="qkv_b_qk")
        vb_d = dram.tile([1, D], BF16, kind="ExternalInput", name="qkv_b_v")
        projw_d = dram.tile([P, KT, D], BF16, kind="ExternalInput", name="proj_w")
        projb_d = dram.tile([1, D], BF16, kind="ExternalInput", name="proj_b")
        fc1w_d = dram.tile([P, KT, FF], BF16, kind="ExternalInput", name="fc1_w")
        fc1b_d = dram.tile([P, FT], F32, kind="ExternalInput", name="fc1_b")
        fc2w_d = dram.tile([P, FT, D], BF16, kind="ExternalInput", name="fc2_w")
        fc2b_d = dram.tile([1, D], BF16, kind="ExternalInput", name="fc2_b")
        out_d = dram.tile([L, D], F32, kind="ExternalOutput", name="out")
        for t, n in [(x_d, "x"), (c_d, "c"), (ln_d, "ln"), (adaw_d, "ada_w"),
                     (adab_d, "ada_b"), (qkvw_d, "qkv_w"), (qkb_d, "qkv_b_qk"),
                     (vb_d, "qkv_b_v"), (projw_d, "proj_w"),
                     (projb_d, "proj_b"), (fc1w_d, "fc1_w"), (fc1b_d, "fc1_b"),
                     (fc2w_d, "fc2_w"), (fc2b_d, "fc2_b"), (out_d, "out")]:
            names[n] = t.name
        out_v = out_d[:].rearrange("(t p) d -> p t d", p=P)

        # ---- persistent tiles ----
        x_sb = per.tile([P, TT, D], F32)
        nc.sync.dma_start(x_sb[:], x_d[:])
        ident = per.tile([P, P], BF16)
        make_identity(nc, ident[:])
        ones_row = per.tile([1, P], BF16)
        nc.vector.memset(ones_row[:], 1.0)
        eps_sb = per.tile([P, 1], F32)
        nc.vector.memset(eps_sb[:], LN_EPS)

        qT = per.tile([P, KT, L], BF16)
        kTt = per.tile([P, KT, L], BF16)
        v_sb = per.tile([P, TT, H, HD + 1], BF16)
        nc.vector.memset(v_sb[:, :, :, HD:HD + 1], 1.0)
        qkb_sb = per.tile([P, 2 * KT], F32)
        fc1b_sb = per.tile([P, FT], F32)
        vb_row = per.tile([1, D], BF16)
        pb_row = per.tile([1, D], BF16)
        f2b_row = per.tile([1, D], BF16)
        zero_row = per.tile([1, 512], BF16)
        nc.vector.memset(zero_row[:], 0.0)

        # modulation broadcast tiles (filled after ada)
        eff1s = per.tile([P, D], BF16)
        eff1h = per.tile([P, D], BF16)
        eff2s = per.tile([P, D], BF16)
        eff2h = per.tile([P, D], BF16)
        gate1 = per.tile([P, D], BF16)
        gate2 = per.tile([P, D], BF16)

        _work_cm = tc.tile_pool(name="work", bufs=2)
        work = _work_cm.__enter__()
        _vp_cm = tc.tile_pool(name="vpart_pool", bufs=1)
        vpp = _vp_cm.__enter__()
        qkvw_vpart = vpp.tile([P, KT, D], BF16)

        def ln_stats(x_aps):
            n = len(x_aps)
            mv = work.tile([P, n, 2], F32, tag=f"mv{n}", name=f"mv{n}")
            for i, x_ap in enumerate(x_aps):
                stats = work.tile([P, 2, 6], F32, tag="stats")
                for sg in range(2):
                    nc.vector.bn_stats(stats[:, sg, :],
                                       x_ap[:, sg * 512:(sg + 1) * 512])
                nc.vector.bn_aggr(mv[:, i, :], stats[:])
            rstd = work.tile([P, n], F32, tag=f"rstd{n}", name=f"rstd{n}")
            nc.scalar.activation(rstd[:], mv[:, :, 1], AF.Sqrt, bias=eps_sb[:])
            nc.vector.reciprocal(rstd[:], rstd[:])
            return mv, rstd

        mv1, rstd1 = ln_stats([x_sb[:, t, :] for t in range(TT)])

        # ---- ada: ssg = silu(c) @ ada_w + ada_b; modulation vectors ----
        with nc.named_scope("ada"), \
                tc.tile_pool(name="ada_sb", bufs=1) as asb, \
                tc.tile_pool(name="ada_stream", bufs=2) as ast, \
                tc.tile_pool(name="ps_ada", bufs=2, space="PSUM") as psa:
            ln_rows = []
            for i in range(4):
                lr = asb.tile([1, D], BF16, name=f"ln_row{i}")
                nc.sync.dma_start(lr[:], ln_d[i:i + 1, :])
                ln_rows.append(lr)
            c_sb = asb.tile([P, KT], F32)
            nc.sync.dma_start(c_sb[:], c_d[:])
            silu_sb = asb.tile([P, KT], BF16)
            nc.scalar.activation(silu_sb[:], c_sb[:], AF.Silu)
            ssg = asb.tile([1, 6 * D], F32)
            t0 = asb.tile([1, D], F32, name="t0")
            t1 = asb.tile([1, D], F32, name="t1")
            r1s = asb.tile([1, D], BF16, name="r1s")
            r1h = asb.tile([1, D], BF16, name="r1h")
            r2s = asb.tile([1, D], BF16, name="r2s")
            r2h = asb.tile([1, D], BF16, name="r2h")
            rg1 = asb.tile([1, D], BF16, name="rg1")
            rg2 = asb.tile([1, D], BF16, name="rg2")
            for nch in range(24):
                aw = ast.tile([P, KT, 256], BF16, tag="aw")
                nc.sync.dma_start(aw[:], adaw_d[:, :, nch * 256:(nch + 1) * 256])
                ab = ast.tile([1, 256], F32, tag="ab")
                nc.sync.dma_start(ab[:], adab_d[:, nch * 256:(nch + 1) * 256])
                pa = psa.tile([1, 256], F32, tag="ada")
                for k in range(KT):
                    nc.tensor.matmul(pa[:], silu_sb[:, k:k + 1], aw[:, k, :],
                                     start=(k == 0), stop=(k == KT - 1))
                nc.vector.tensor_add(ssg[:, nch * 256:(nch + 1) * 256], pa[:],
                                     ab[:])
                if nch == 11:
                    # msa vectors ready: unblock LN1 modulate early
                    nc.vector.tensor_scalar_add(t0[:], ssg[:, D:2 * D], 1.0)
                    nc.vector.tensor_mul(r1s[:], t0[:], ln_rows[0][:])
                    nc.vector.tensor_mul(t1[:], t0[:], ln_rows[1][:])
                    nc.vector.tensor_add(r1h[:], t1[:], ssg[:, 0:D])
                    nc.vector.tensor_copy(rg1[:], ssg[:, 2 * D:3 * D])
                    nc.gpsimd.partition_broadcast(eff1s[:], r1s[:])
                    nc.gpsimd.partition_broadcast(eff1h[:], r1h[:])
                    nc.gpsimd.partition_broadcast(gate1[:], rg1[:])
                    # queue the V weights + first QKV weight chunks now so the
                    # QKV phase isn't starved behind the rest of ada_w
                    nc.sync.dma_start(qkvw_vpart[:], qkvw_d[:, :, 2 * D:3 * D])
            # mlp: shift [3D:4D], scale [4D:5D], gate [5D:6D]
            nc.vector.tensor_scalar_add(t0[:], ssg[:, 4 * D:5 * D], 1.0)
            nc.vector.tensor_mul(r2s[:], t0[:], ln_rows[2][:])
            nc.vector.tensor_mul(t1[:], t0[:], ln_rows[3][:])
            nc.vector.tensor_add(r2h[:], t1[:], ssg[:, 3 * D:4 * D])
            nc.vector.tensor_copy(rg2[:], ssg[:, 5 * D:6 * D])
            nc.gpsimd.partition_broadcast(eff2s[:], r2s[:])
            nc.gpsimd.partition_broadcast(eff2h[:], r2h[:])
            nc.gpsimd.partition_broadcast(gate2[:], rg2[:])

        # persistent weight/bias DMAs (after the ada stream in queue order)
        nc.sync.dma_start(qkb_sb[:], qkb_d[:])
        nc.sync.dma_start(fc1b_sb[:], fc1b_d[:])
        nc.sync.dma_start(vb_row[:], vb_d[:])
        nc.sync.dma_start(pb_row[:], projb_d[:])
        nc.sync.dma_start(f2b_row[:], fc2b_d[:])

        if True:

            def layernorm_modulate(x_ap, mv, rstd, i, effs, effh, h_out):
                norm = work.tile([P, D], BF16, tag="norm", bufs=1)
                nc.vector.tensor_scalar(out=norm[:], in0=x_ap,
                                        scalar1=mv[:, i, 0:1],
                                        scalar2=rstd[:, i:i + 1], op0=OP.subtract,
                                        op1=OP.mult)
                nc.vector.tensor_mul(h_out, norm[:], effs[:])
                nc.vector.tensor_add(h_out, h_out, effh[:])

            def transpose_to(src_bf16, dst_ap_fn):
                for k in range(KT):
                    pt = ps_t.tile([P, P], BF16, tag="transp")
                    nc.tensor.transpose(pt[:], src_bf16[:, k * P:(k + 1) * P],
                                        ident[:])
                    nc.vector.tensor_copy(dst_ap_fn(k), pt[:])

            # ---- LN1 + modulate + transpose -> h1T; QKV ----
            with nc.named_scope("qkv"), \
                    tc.tile_pool(name="qkv_sb", bufs=1) as qsb, \
                    tc.tile_pool(name="qkv_stream", bufs=3) as qst, \
                    tc.tile_pool(name="ps_qkv", bufs=2, space="PSUM") as psq:
                h1T = qsb.tile([P, KT, L], BF16)
                for t in range(TT):
                    h1 = qst.tile([P, D], BF16, tag="h1")
                    layernorm_modulate(x_sb[:, t, :], mv1, rstd1, t,
                                       eff1s, eff1h, h1[:])
                    transpose_to(h1[:], lambda k, _t=t: h1T[:, k, _t * P:(_t + 1) * P])
                for t in range(TT):
                    for vh in range(2):
                        pv = psq.tile([P, 512], F32, tag="pv")
                        for k in range(KT):
                            nc.tensor.matmul(pv[:], h1T[:, k, t * P:(t + 1) * P],
                                             qkvw_vpart[:, k, vh * 512:(vh + 1) * 512],
                                             start=(k == 0), stop=False)
                        nc.tensor.matmul(pv[:], ones_row[:],
                                         vb_row[:, vh * 512:(vh + 1) * 512],
                                         start=False, stop=True)
                        for hh in range(8):
                            h = vh * 8 + hh
                            nc.vector.tensor_copy(v_sb[:, t, h, 0:HD],
                                                  pv[:, hh * HD:(hh + 1) * HD])
                for ji in range(2 * KT):  # Q0,K0,Q1,K1,... chunk order
                    jj = ji // 2
                    j = jj + (KT if ji % 2 else 0)
                    wj = qst.tile([P, KT, P], BF16, tag="wj")
                    nc.sync.dma_start(wj[:], qkvw_d[:, :, j * P:(j + 1) * P])
                    dst = qT if j < KT else kTt
                    for th in range(2):
                        pq = psq.tile([P, 512], F32, tag="pqk")
                        for k in range(KT):
                            nc.tensor.matmul(pq[:], wj[:, k, :],
                                             h1T[:, k, th * 512:(th + 1) * 512],
                                             start=(k == 0), stop=(k == KT - 1))
                        nc.vector.tensor_scalar_add(
                            dst[:, jj, th * 512:(th + 1) * 512], pq[:],
                            qkb_sb[:, j:j + 1])

            _vp_cm.__exit__(None, None, None)

            # ---- attention + MLP, software-pipelined over query chunks ----
            with tc.tile_pool(name="attn", bufs=2) as ap, \
                    tc.tile_pool(name="mlp", bufs=1) as mp, \
                    tc.tile_pool(name="fc_stream", bufs=3) as fs, \
                    tc.tile_pool(name="ps_m", bufs=2, space="PSUM") as psm:

                def attention_pair(qc, j, aT):
                    """scores + exp + attn@V + normalize for heads (2j, 2j+1)."""
                    q0 = qc * QC
                    attnA = ap.tile([P, TT, QC], BF16, tag="attnA")
                    attnB = ap.tile([P, TT, QC], BF16, tag="attnB")
                    for mg in range(TT // 2):  # m-pairs share one psum bank
                        psA = psm.tile([P, 2 * QC], F32, tag="scores", bufs=2,
                                       name="psA")
                        psB = psm.tile([P, 2 * QC], F32, tag="scores", bufs=2,
                                       name="psB")
                        for mi in range(2):
                            m = 2 * mg + mi
                            nc.tensor.matmul(psA[:, mi * QC:(mi + 1) * QC],
                                             kTt[0:HD, j, m * P:(m + 1) * P],
                                             qT[0:HD, j, q0:q0 + QC],
                                             start=True, stop=True)
                            nc.tensor.matmul(psB[:, mi * QC:(mi + 1) * QC],
                                             kTt[HD:P, j, m * P:(m + 1) * P],
                                             qT[HD:P, j, q0:q0 + QC],
                                             start=True, stop=True,
                                             tile_position=(HD, 0))
                        nc.scalar.activation(attnA[:, 2 * mg:2 * mg + 2, :],
                                             psA[:], AF.Exp, scale=0.125)
                        nc.scalar.activation(attnB[:, 2 * mg:2 * mg + 2, :],
                                             psB[:], AF.Exp, scale=0.125)
                    puA = psm.tile([HD + 1, QC], F32, tag="pu", bufs=2, name="puA")
                    puB = psm.tile([HD + 1, QC], F32, tag="pu", bufs=2, name="puB")
                    for m in range(TT):
                        # attn@V with the ones column fused in lhsT: psum row
                        # 64 accumulates the softmax denominator for free
                        nc.tensor.matmul(puA[:], v_sb[:, m, 2 * j, :],
                                         attnA[:, m, :],
                                         start=(m == 0), stop=(m == TT - 1))
                        nc.tensor.matmul(puB[:], v_sb[:, m, 2 * j + 1, :],
                                         attnB[:, m, :],
                                         start=(m == 0), stop=(m == TT - 1))
                    recA = ap.tile([1, QC], F32, tag="recA", bufs=1)
                    recB = ap.tile([1, QC], F32, tag="recB", bufs=1)
                    nc.vector.tensor_copy(recA[:], puA[HD:HD + 1, :])
                    nc.vector.tensor_copy(recB[:], puB[HD:HD + 1, :])
                    rbA = ap.tile([HD, QC], F32, tag="rbA", bufs=1)
                    rbB = ap.tile([HD, QC], F32, tag="rbB", bufs=1)
                    nc.gpsimd.partition_broadcast(rbA[:], recA[:])
                    nc.gpsimd.partition_broadcast(rbB[:], recB[:])
                    nc.vector.reciprocal_approx_fast(rbA[:], rbA[:])
                    nc.vector.reciprocal_approx_fast(rbB[:], rbB[:])
                    nc.vector.tensor_mul(aT[0:HD, j, :], puA[0:HD, :], rbA[:])
                    nc.vector.tensor_mul(aT[HD:P, j, :], puB[0:HD, :], rbB[:])

                def proj_blocks(qc, aT):
                    pwt = {}

                    def load_pw():
                        pw = fs.tile([P, KT, D], BF16, tag="pw", bufs=1)
                        nc.sync.dma_start(pw[:], projw_d[:])
                        pwt["pw"] = pw
                    yield load_pw
                    for s in range(QSUB):
                        t = qc * QSUB + s

                        def proj_block(s=s, t=t):
                            pw = pwt["pw"]
                            for nh in range(2):
                                pp = psm.tile([P, 512], F32, tag="pu", bufs=2,
                                              name="pp")
                                for k in range(KT):
                                    nc.tensor.matmul(
                                        pp[:], aT[:, k, s * P:(s + 1) * P],
                                        pw[:, k, nh * 512:(nh + 1) * 512],
                                        start=(k == 0), stop=False)
                                nc.tensor.matmul(
                                    pp[:], ones_row[:],
                                    pb_row[:, nh * 512:(nh + 1) * 512],
                                    start=False, stop=True)
                                tmp = work.tile([P, 512], F32, tag="etmp", bufs=1)
                                nc.vector.tensor_mul(
                                    tmp[:], pp[:],
                                    gate1[:, nh * 512:(nh + 1) * 512])
                                nc.vector.tensor_add(
                                    x_sb[:, t, nh * 512:(nh + 1) * 512],
                                    x_sb[:, t, nh * 512:(nh + 1) * 512], tmp[:])
                        yield proj_block

                def ln2_blocks(qc, h2T):
                    state = {}

                    def stats_block():
                        state["mv"], state["rstd"] = ln_stats(
                            [x_sb[:, qc * QSUB + s, :] for s in range(QSUB)])
                    yield stats_block
                    for s in range(QSUB):
                        def ln2_block(s=s):
                            t = qc * QSUB + s
                            h2 = work.tile([P, D], BF16, tag="h2", bufs=1)
                            layernorm_modulate(x_sb[:, t, :], state["mv"],
                                               state["rstd"], s,
                                               eff2s, eff2h, h2[:])
                            transpose_to(h2[:],
                                         lambda k, _s=s: h2T[:, k,
                                                             _s * P:(_s + 1) * P])
                        yield ln2_block

                def mlp_blocks(qc, h2T, geluT):
                    """MLP sub-blocks for chunk qc, drained during the next
                    chunk's attention to keep the PE dense."""
                    fc1_list = []
                    fc2_list = []
                    for fg in range(FT // 4):  # 8 blocks of 4 ff-chunks
                        def fc1_block(fg=fg):
                            for fi in range(4):
                                fc = 4 * fg + fi
                                w1 = fs.tile([P, KT, P], BF16, tag="w1", bufs=2)
                                nc.sync.dma_start(
                                    w1[:], fc1w_d[:, :, fc * P:(fc + 1) * P])
                                pf = ps_t.tile([P, QC], F32, tag="transp",
                                               name="pf")
                                for k in range(KT):
                                    nc.tensor.matmul(pf[:], w1[:, k, :],
                                                     h2T[:, k, :],
                                                     start=(k == 0),
                                                     stop=(k == KT - 1))
                                # DVE eviction with fused bias add
                                nc.vector.tensor_scalar_add(
                                    geluT[:, fc, :], pf[:],
                                    fc1b_sb[:, fc:fc + 1])
                            # medium-grained gelu (avoids one huge ACT convoy
                            # and per-op table reloads alike)
                            nc.scalar.activation(
                                geluT[:, 4 * fg:4 * fg + 4, :],
                                geluT[:, 4 * fg:4 * fg + 4, :],
                                AF.Gelu_apprx_tanh)
                        fc1_list.append(fc1_block)

                    # fc2 with half-resident weights: each 4MB half loaded once
                    # per chunk and reused for both token sub-tiles
                    FH = FT // 4  # 8 ff tiles per quarter
                    state = {}
                    for ffh in range(4):
                        for s in range(QSUB):
                            def fc2_block(ffh=ffh, s=s):
                                t = qc * QSUB + s
                                if s == 0:
                                    w2h = fs.tile([P, FH, D], BF16, tag="w2h",
                                                  bufs=1)
                                    nc.sync.dma_start(
                                        w2h[:],
                                        fc2w_d[:, FH * ffh:FH * (ffh + 1), :])
                                    state["w2h"] = w2h
                                w2h = state["w2h"]
                                pf2 = psm.tile([P, D], F32, tag="fc2", bufs=1,
                                               name="pf2")
                                for fi in range(FH):
                                    ft = FH * ffh + fi
                                    for nh in range(2):
                                        nc.tensor.matmul(
                                            pf2[:, nh * 512:(nh + 1) * 512],
                                            geluT[:, ft, s * P:(s + 1) * P],
                                            w2h[:, fi, nh * 512:(nh + 1) * 512],
                                            start=(fi == 0),
                                            stop=(ffh > 0 and fi == FH - 1))
                                if ffh < 3:
                                    if ffh == 0:
                                        for nh in range(2):
                                            nc.tensor.matmul(
                                                pf2[:, nh * 512:(nh + 1) * 512],
                                                ones_row[:],
                                                f2b_row[:, nh * 512:(nh + 1) * 512],
                                                start=False, stop=True)
                                        acc = work.tile([P, D], F32,
                                                        tag=f"facc{s}",
                                                        name=f"facc{s}")
                                        state[f"acc{s}"] = acc
                                        nc.vector.tensor_copy(acc[:], pf2[:])
                                    else:
                                        acc = state[f"acc{s}"]
                                        nc.vector.tensor_add(acc[:], acc[:],
                                                             pf2[:])
                                else:
                                    acc = state[f"acc{s}"]
                                    ot = work.tile([P, D], F32, tag="ot",
                                                   bufs=1)
                                    nc.vector.tensor_add(acc[:], acc[:], pf2[:])
                                    nc.vector.tensor_mul(acc[:], acc[:],
                                                         gate2[:])
                                    nc.vector.tensor_add(ot[:], acc[:],
                                                         x_sb[:, t, :])
                                    nc.sync.dma_start(out_v[:, t, :], ot[:])
                            fc2_list.append(fc2_block)

                    # interleave: fc2 block after every 2nd fc1 block
                    order = []
                    fi1 = fi2 = 0
                    for fg in range(FT // 4):
                        order.append(fc1_list[fg])
                        if fg >= 1 and fg % 1 == 0 and fi2 < len(fc2_list) and fg * 4 >= (fi2 // 2 + 1) * (FT // 4):
                            pass
                    # simple static interleave: f0 f1 F0 f2 F1 f3 F2 f4 F3 f5 F4 f6 F5 f7 F6 F7
                    seq = [fc1_list[0], fc1_list[1], fc2_list[0], fc1_list[2],
                           fc2_list[1], fc1_list[3], fc2_list[2], fc1_list[4],
                           fc2_list[3], fc1_list[5], fc2_list[4], fc1_list[6],
                           fc2_list[5], fc1_list[7], fc2_list[6], fc2_list[7]]
                    for blk in seq:
                        yield blk

                pending = deque()
                for qc in range(NQC):
                    with nc.named_scope(f"attn{qc}"):
                        aT = ap.tile([P, KT, QC], BF16, tag="aT", name=f"aT{qc}", bufs=1)
                        for j in range(KT):
                            attention_pair(qc, j, aT)
                            if j % 2 == 1:
                                for _ in range(4):
                                    if pending:
                                        pending.popleft()()
                        for blk in proj_blocks(qc, aT):
                            blk()
                    with nc.named_scope(f"mlp{qc}"):
                        h2T = mp.tile([P, KT, QC], BF16, tag="h2T",
                                      name=f"h2T{qc}")
                        geluT = mp.tile([P, FT, QC], BF16, tag="geluT",
                                        name=f"geluT{qc}")
                        for blk in ln2_blocks(qc, h2T):
                            blk()
                        pending.extend(mlp_blocks(qc, h2T, geluT))
                while pending:
                    pending.popleft()()

        _work_cm.__exit__(None, None, None)

    nc.compile()
    return nc, names


def _get_compiled():
    if "nc" not in _CACHE:
        _CACHE["nc"], _CACHE["names"] = _build()
    return _CACHE["nc"], _CACHE["names"]


def kernel(x, c, ln1_w, ln1_b, ln2_w, ln2_b, ada_w, ada_b,
           qkv_w, qkv_b, proj_w, proj_b, fc1_w, fc1_b, fc2_w, fc2_b,
           _trace=False):
    nc, names = _get_compiled()
    x = np.asarray(x, dtype=np.float32)
    c = np.asarray(c, dtype=np.float32)
    ln = np.stack([np.asarray(a) for a in (ln1_w, ln1_b, ln2_w, ln2_b)]).astype(BF)

    def pmajor(w):
        # [K*P, N] -> [P, K, N] with row k*P+p at [p, k]
        w = np.asarray(w)
        kp, n = w.shape
        return np.ascontiguousarray(
            w.reshape(kp // P, P, n).transpose(1, 0, 2)).astype(BF)

    qkv_b = np.asarray(qkv_b, np.float32)
    common = {
        names["ln"]: ln,
        names["ada_w"]: pmajor(ada_w),
        names["ada_b"]: np.asarray(ada_b, np.float32).reshape(1, -1),
        names["qkv_w"]: pmajor(qkv_w),
        names["qkv_b_qk"]: np.ascontiguousarray(
            qkv_b[0:2 * D].reshape(2 * KT, P).T),
        names["qkv_b_v"]: qkv_b[2 * D:3 * D].astype(BF).reshape(1, -1),
        names["proj_w"]: pmajor(proj_w),
        names["proj_b"]: np.asarray(proj_b).astype(BF).reshape(1, -1),
        names["fc1_w"]: pmajor(fc1_w),
        names["fc1_b"]: np.ascontiguousarray(
            np.asarray(fc1_b, np.float32).reshape(FT, P).T),
        names["fc2_w"]: pmajor(fc2_w),
        names["fc2_b"]: np.asarray(fc2_b).astype(BF).reshape(1, -1),
    }
    in_maps = []
    for b in range(B):
        m = dict(common)
        m[names["x"]] = np.ascontiguousarray(
            x[b].reshape(TT, P, D).transpose(1, 0, 2))
        m[names["c"]] = np.ascontiguousarray(c[b].reshape(KT, P).T)
        in_maps.append(m)
    res = bass_utils.run_bass_kernel_spmd(nc, in_maps, core_ids=list(range(B)),
                                          trace=_trace)
    out = np.stack([res.results[b][names["out"]] for b in range(B)])
    if _trace:
        _CACHE["last_result"] = res
    return out
